# revision 22
# baseline (speedup 1.0000x reference)
"""ADMM deconvolution (DiffuserCam-style) Trainium2 kernel.

kernel(**inputs) takes FULL inputs, returns FULL output [1,3,486,648].
Channel-parallel: core c in {0,1,2} computes channel c; cores 3..7 run the
same SPMD program on zero data. One tiny AllReduce(max) for the final
per-image normalization.

FFT2 is computed as TensorEngine matmul stages (fp32r) with layouts chosen so
no stage needs an explicit transpose:
  spatial  [972(n)->8 tiles, 1296(w) free]
  CT(A->B) [1296(w)->11 tiles, 488(k) free] x {re,im}   (k 487 is a zero pad)
  freq     [1296(l)->11 tiles, 488(k) free] x {re,im}
  W        [488(k)->4 chunks, 432(w) free] x {re,im}  (one 432-col chunk live)
The 1/(HG*WG) ifft scale is folded into Smult. ADMM dual state is folded:
p1 = a21/m2 - u1 (so d1 = -p1) and likewise p2; a21/a22/u1/u2 never exist.
Binv+Ainv+state-update are fused per 432-column chunk, so skp/Hskp round-trip
neither DRAM nor full-grid SBUF.

SBUF: three rotating 43KB/partition "big grid" slots (tags slotA/B/C),
phase-local pools for everything else.
"""

import sys

sys.path.insert(0, "/opt/trn_rl_repo")

import os
import numpy as np

D0, D1 = 486, 648
P0, P1 = D0 // 2, D1 // 2
HG, WG = 2 * D0, 2 * D1  # 972 x 1296
KH = HG // 2 + 1  # 487
KHP = KH + 1  # 488 (even; col 487 always zero)
K2 = 2 * KHP
ITERS = int(os.environ.get("K_ITERS", "10"))
DEBUG_SK = os.environ.get("K_DEBUG_SK", "0") == "1"

NT = [(i * 128, min(128, HG - i * 128)) for i in range((HG + 127) // 128)]  # 8
WT = [(i * 128, min(128, WG - i * 128)) for i in range((WG + 127) // 128)]  # 11
KC = [(i * 122, 122) for i in range(4)]  # uniform k-chunks (488 = 4*122)
WC3 = [(i * 432, 432) for i in range(3)]
SCALE = float(HG * WG)
CROP = [(P0 + i * 128, min(128, D0 - i * 128)) for i in range((D0 + 127) // 128)]  # 4


def _make_consts():
    n = np.arange(HG)
    k = np.arange(KH)
    w = np.arange(WG)
    ang_h = 2 * np.pi * np.outer(n, k) / HG
    fhr = np.zeros((HG, KHP), np.float32)
    fhi = np.zeros((HG, KHP), np.float32)
    fhr[:, :KH] = np.cos(ang_h)
    fhi[:, :KH] = -np.sin(ang_h)
    ang_w = 2 * np.pi * np.outer(w, w) / WG
    cw = np.cos(ang_w).astype(np.float32)
    sp = np.sin(ang_w).astype(np.float32)
    sn = (-sp).astype(np.float32)
    ck = np.full(KH, 2.0)
    ck[0] = 1.0
    ck[KH - 1] = 1.0
    ang_a = 2 * np.pi * np.outer(k, n) / HG
    atr = np.zeros((KHP, HG), np.float32)
    ati = np.zeros((KHP, HG), np.float32)
    atr[:KH] = ck[:, None] * np.cos(ang_a)
    ati[:KH] = -ck[:, None] * np.sin(ang_a)
    lapl = np.zeros((HG, WG), np.float32)
    lapl[0, 0] = 4.0
    lapl[0, 1] = lapl[1, 0] = lapl[0, -1] = lapl[-1, 0] = -1.0
    ltl_t = np.zeros((WG, KHP), np.float32)
    ltl_t[:, :KH] = np.abs(np.fft.fft2(lapl))[:KH, :].T.astype(np.float32)
    return fhr, fhi, cw, sp, sn, atr, ati, ltl_t


def _build(scalars):
    """scalars: list of (m1, m2, m3, tau) python floats, one per iteration."""
    import concourse.mybir as mybir
    from concourse import bacc
    from concourse.tile import TileContext

    F32 = mybir.dt.float32
    AF = mybir.ActivationFunctionType
    OP = mybir.AluOpType
    n_iters = len(scalars)

    sm_keys, sm_idx, vm_keys, vm_idx = [], [], [], []
    for m1, m2, m3, _ in scalars:
        key = (m1, m2, m3)
        if key not in sm_keys:
            sm_keys.append(key)
        sm_idx.append(sm_keys.index(key))
        if m1 not in vm_keys:
            vm_keys.append(m1)
        vm_idx.append(vm_keys.index(m1))

    nc = bacc.Bacc(None, num_devices=8)

    # register const APs for activation biases (ACT needs [128,1] AP biases)
    cvals = set()
    for m1, m2, m3, tau in scalars:
        cvals.update((-tau, tau, m3 * SCALE))
    for v in sorted(cvals):
        if (mybir.dt.float32, v) not in nc.const_aps.aps:
            t = nc.alloc_sbuf_tensor(f"cst-{v!r}", [128, 1], mybir.dt.float32)
            nc.gpsimd.memset(t.ap(), v)
            nc.const_aps.aps[(mybir.dt.float32, v)] = t.ap()
    nc.all_engine_barrier()

    MDT = mybir.dt.float32r if os.environ.get("K_MM_DT", "f32r") == "f32r" else F32
    FDT = mybir.dt.bfloat16 if os.environ.get("K_FREQ_BF16", "0") == "1" else MDT

    def rd(ap):
        # freq tiles read by non-matmul engines
        return ap.bitcast(F32) if FDT == mybir.dt.float32r else ap

    cty = nc.dram_tensor("cty", [HG, WG], F32, kind="ExternalInput")
    hps = nc.dram_tensor("hps", [HG, WG], MDT, kind="ExternalInput")
    vmult = nc.dram_tensor("vmult", [len(vm_keys), HG, WG], F32, kind="ExternalInput")
    fhr = nc.dram_tensor("fhr", [HG, KHP], MDT, kind="ExternalInput")
    fhi = nc.dram_tensor("fhi", [HG, KHP], MDT, kind="ExternalInput")
    mats = nc.dram_tensor("mats", [3, WG, WG], FDT, kind="ExternalInput")
    atm = nc.dram_tensor("atm", [2, KHP, HG], MDT, kind="ExternalInput")
    ltl = nc.dram_tensor("ltl", [WG, KHP], F32, kind="ExternalInput")
    out = nc.dram_tensor("out", [D0, D1], F32, kind="ExternalOutput")
    dbg = None
    if DEBUG_SK:
        dbg = nc.dram_tensor("dbg", [HG, WG], F32, kind="ExternalOutput")

    def mm(ps, lhsT, rhs, start, stop):
        nc.tensor.matmul(ps, lhsT, rhs, start=start, stop=stop)

    with TileContext(nc, pool_alloc_mode="queue") as tc:
        dram = tc.alloc_tile_pool(name="dstate", bufs=1, space="DRAM")
        hb_d = dram.tile([2, WG, KHP], F32, name="hb_d")
        sm_d = dram.tile([len(sm_keys), WG, KHP], F32, name="sm_d")
        st_d = [
            dram.tile([4, HG, WG], F32, name=f"st{i}") for i in range(n_iters + 1)
        ]  # planes: 0=a1, 1=a3, 2=sk, 3=vkp
        p1_d = [dram.tile([HG + 1, WG], F32, name=f"p1_{i}") for i in range(2)]
        p2_d = [dram.tile([HG, WG + 1], F32, name=f"p2_{i}") for i in range(2)]
        ccin = dram.tile([1, 1], F32, name="ccin")
        ccout = dram.tile([1, 1], F32, name="ccout", addr_space="Shared")

        big = tc.alloc_tile_pool(name="big", bufs=1)
        _slot_n = [0]

        def big_tile(shape, slot, dt=None):
            _slot_n[0] += 1
            return big.tile(
                shape, dt or MDT, name=f"bt{_slot_n[0]}", tag=slot, bufs=1
            )

        # ---------------- emitters ----------------

        def load_fh(fp):
            """[128, 8, 2, KHP] resident copy of fhr/fhi for a whole iteration"""
            fht = fp.tile([128, 8, 2, KHP], MDT, name="fhres")
            nc.sync.dma_start(
                out=fht[:, 0:7, 0],
                in_=fhr[:896].rearrange("(t p) k -> p t k", p=128),
            )
            nc.sync.dma_start(out=fht[:76, 7, 0], in_=fhr[896:])
            nc.sync.dma_start(
                out=fht[:, 0:7, 1],
                in_=fhi[:896].rearrange("(t p) k -> p t k", p=128),
            )
            nc.sync.dma_start(out=fht[:76, 7, 1], in_=fhi[896:])
            return fht

        def stageA(grid, ct, fht):
            """forward height-rDFT: grid [128,8,1296] -> ct [128,11,K2]"""
            for g0 in range(0, 11, 4):
                grp = [wc for wc in range(g0, min(g0 + 4, 11))]
                with tc.tile_pool(name="psa", bufs=1, space="PSUM") as pp:
                    pss = {}
                    for wc in grp:
                        for pl in range(2):
                            pss[(wc, pl)] = pp.tile(
                                [128, KHP], F32, name=f"ps_{wc}_{pl}"
                            )
                    for nt, (r0, p) in enumerate(NT):
                        for wc in grp:
                            w0, wsz = WT[wc]
                            for pl in range(2):
                                mm(
                                    pss[(wc, pl)][:wsz],
                                    grid[:p, nt, w0 : w0 + wsz],
                                    fht[:p, nt, pl],
                                    start=(nt == 0),
                                    stop=(nt == len(NT) - 1),
                                )
                    for wc in grp:
                        w0, wsz = WT[wc]
                        for pl in range(2):
                            nc.scalar.copy(
                                ct[:wsz, wc, pl * KHP : (pl + 1) * KHP],
                                pss[(wc, pl)][:wsz],
                            )

        def stageB(cts, sink):
            """width-DFT for each ct in cts (shared weight streams); sink
            consumes {(ti, pl): psum} per l-chunk."""
            psb_bufs = int(os.environ.get("K_SB_BUFS", "2"))
            with (
                tc.tile_pool(name="cbp", bufs=psb_bufs) as cbp,
                tc.tile_pool(name="psb", bufs=psb_bufs, space="PSUM") as pp,
            ):
                for lc in range(11):
                    l0, lsz = WT[lc]
                    cb = cbp.tile([128, 3, 11, 128], FDT, name="cb", tag="cb")
                    for mi in range(3):
                        bulk = mats[mi, :1280, l0 : l0 + lsz].rearrange(
                            "(wt p) j -> p wt j", p=128
                        )
                        nc.sync.dma_start(out=cb[:, mi, :10, :lsz], in_=bulk)
                        nc.sync.dma_start(
                            out=cb[:16, mi, 10, :lsz],
                            in_=mats[mi, 1280:, l0 : l0 + lsz],
                        )
                    pss = {}
                    for ti in range(len(cts)):
                        for pl in range(2):
                            pss[(ti, pl)] = pp.tile(
                                [128, KHP], F32, name=f"psb_{ti}_{pl}",
                                tag=f"psbt_{ti}_{pl}",
                            )
                    for wt in range(11):
                        w0, ksz = WT[wt]
                        for ti, ct in enumerate(cts):
                            # Yr += cw.T@Cr + sp.T@Ci ; Yi += sn.T@Cr + cw.T@Ci
                            for pl, (ma, mb) in enumerate(((0, 1), (2, 0))):
                                mm(
                                    pss[(ti, pl)][:lsz],
                                    cb[:ksz, ma, wt, :lsz],
                                    ct[:ksz, wt, 0:KHP],
                                    start=(wt == 0),
                                    stop=False,
                                )
                                mm(
                                    pss[(ti, pl)][:lsz],
                                    cb[:ksz, mb, wt, :lsz],
                                    ct[:ksz, wt, KHP:K2],
                                    start=False,
                                    stop=(wt == 10),
                                )
                    sink(lc, lsz, pss)

        def pointwise_T(it, that, has_s, ytv=None):
            """sink for stageB: T = Smult*(S + m1*conj(H)*V) into `that`.
            V planes come from psum (merged) or from the ytv SBUF tile."""
            m1 = scalars[it][0]
            smi = sm_idx[it]

            def sink(lc, lsz, pss):
                l0 = WT[lc][0]
                with (
                    tc.tile_pool(name="hstp", bufs=2) as hp,
                    tc.tile_pool(name="twp", bufs=1) as twp,
                ):
                    hst = hp.tile([128, 3, KHP], F32, name="hst", tag="hst")
                    nc.sync.dma_start(
                        out=hst[:lsz, 0:2],
                        in_=hb_d[:, l0 : l0 + lsz].rearrange("m p k -> p m k"),
                    )
                    nc.sync.dma_start(out=hst[:lsz, 2], in_=sm_d[smi, l0 : l0 + lsz])
                    hr = hst[:lsz, 0]
                    hi = hst[:lsz, 1]
                    smt = hst[:lsz, 2]
                    if ytv is not None:
                        vr = rd(ytv[:lsz, lc, 0:KHP])
                        vi = rd(ytv[:lsz, lc, KHP:K2])
                    else:
                        iv = 1 if has_s else 0
                        vr = pss[(iv, 0)][:lsz]
                        vi = pss[(iv, 1)][:lsz]
                    tw = twp.tile([128, 4, KHP], F32, name="tw", tag="tw")
                    c1 = tw[:lsz, 0]
                    c2 = tw[:lsz, 1]
                    c3 = tw[:lsz, 2]
                    c4 = tw[:lsz, 3]
                    trr = that[:lsz, lc, 0:KHP]
                    tii = that[:lsz, lc, KHP:K2]
                    nc.vector.tensor_mul(c1, hr, vr)
                    nc.vector.tensor_mul(c2, hi, vi)
                    nc.vector.tensor_mul(c3, hr, vi)
                    nc.vector.tensor_mul(c4, hi, vr)
                    nc.gpsimd.tensor_add(c1, c1, c2)  # re(conj(H)V)
                    nc.gpsimd.tensor_tensor(
                        c3, c3, c4, op=OP.subtract
                    )  # im(conj(H)V)
                    if has_s:
                        nc.vector.scalar_tensor_tensor(
                            c1, c1, m1, pss[(0, 0)][:lsz], op0=OP.mult, op1=OP.add
                        )
                        nc.vector.scalar_tensor_tensor(
                            c3, c3, m1, pss[(0, 1)][:lsz], op0=OP.mult, op1=OP.add
                        )
                    else:
                        nc.scalar.mul(c1, c1, m1)
                        nc.scalar.mul(c3, c3, m1)
                    nc.gpsimd.tensor_mul(trr, c1, smt)
                    nc.gpsimd.tensor_mul(tii, c3, smt)

            return sink

        def y_sink(ytv):
            def sink(lc, lsz, pss):
                nc.scalar.copy(ytv[:lsz, lc, 0:KHP], pss[(0, 0)][:lsz])
                nc.scalar.copy(ytv[:lsz, lc, KHP:K2], pss[(0, 1)][:lsz])

            return sink

        def emit_G(that, ghat):
            """ghat = H * that (complex pointwise)."""
            for lc in range(11):
                l0, lsz = WT[lc]
                with (
                    tc.tile_pool(name="hg", bufs=2) as hp,
                    tc.tile_pool(name="twg", bufs=2) as twp,
                ):
                    hst = hp.tile([128, 2, KHP], F32, name="hst2", tag="hst2")
                    nc.sync.dma_start(
                        out=hst[:lsz],
                        in_=hb_d[:, l0 : l0 + lsz].rearrange("m p k -> p m k"),
                    )
                    hr = hst[:lsz, 0]
                    hi = hst[:lsz, 1]
                    trr = rd(that[:lsz, lc, 0:KHP])
                    tii = rd(that[:lsz, lc, KHP:K2])
                    grr = ghat[:lsz, lc, 0:KHP]
                    gii = ghat[:lsz, lc, KHP:K2]
                    tw = twp.tile([128, 2, KHP], F32, name="twg", tag="twg")
                    c1 = tw[:lsz, 0]
                    c2 = tw[:lsz, 1]
                    nc.vector.tensor_mul(c1, hr, trr)
                    nc.gpsimd.tensor_mul(c2, hi, tii)
                    nc.vector.tensor_sub(grr, c1, c2)
                    nc.vector.tensor_mul(c1, hr, tii)
                    nc.gpsimd.tensor_mul(c2, hi, trr)
                    nc.vector.tensor_add(gii, c1, c2)

        def binv_ainv_pass7(it, that, ghat, vd_next):
            """Per 432-col chunk: inverse width-DFT (Binv) into a W chunk,
            inverse height-rDFT (Ainv), then the ADMM state updates."""
            m1, m2, m3, _ = scalars[it]
            last = it == n_iters - 1
            m1n = scalars[it + 1][0] if not last else None
            ntr = 1 if last else 2
            srcs = [that] + ([] if last else [ghat])
            wcp = tc.alloc_tile_pool(name="wchunk", bufs=1)
            for wc, (w0, wsz) in enumerate(WC3):
                w_c = wcp.tile(
                    [128, 2, 2, 4, 432], MDT, name=f"wc{it}_{wc}", tag="wc", bufs=1
                )  # [k-in-chunk, tr, pl, kc, w]
                for kh in range(2):
                    kcs = [kh * 2, kh * 2 + 1]
                    with (
                        tc.tile_pool(name="rvp", bufs=2) as rvp,
                        tc.tile_pool(name="psi", bufs=1, space="PSUM") as pp,
                    ):
                        pss = {}
                        for ti in range(ntr):
                            for kc in kcs:
                                for pl in range(2):
                                    pss[(ti, kc, pl)] = pp.tile(
                                        [128, 432], F32, name=f"psi{ti}_{kc}_{pl}"
                                    )
                        for lt in range(11):
                            l0, lsz = WT[lt]
                            rv = rvp.tile([128, 3, 432], FDT, name="rv", tag="rv")
                            nc.sync.dma_start(
                                out=rv[:lsz],
                                in_=mats[:, l0 : l0 + lsz, w0 : w0 + wsz].rearrange(
                                    "m p j -> p m j"
                                ),
                            )
                            for ti, src in enumerate(srcs):
                                for kc in kcs:
                                    k0, ksz = KC[kc]
                                    tr_r = src[:lsz, lt, k0 : k0 + ksz]
                                    tr_i = src[:lsz, lt, KHP + k0 : KHP + k0 + ksz]
                                    # Wr = Tr@cw + Ti@sn ; Wi = Tr@sp + Ti@cw
                                    mm(pss[(ti, kc, 0)][:ksz], tr_r, rv[:lsz, 0],
                                       start=(lt == 0), stop=False)
                                    mm(pss[(ti, kc, 0)][:ksz], tr_i, rv[:lsz, 2],
                                       start=False, stop=(lt == 10))
                                    mm(pss[(ti, kc, 1)][:ksz], tr_r, rv[:lsz, 1],
                                       start=(lt == 0), stop=False)
                                    mm(pss[(ti, kc, 1)][:ksz], tr_i, rv[:lsz, 0],
                                       start=False, stop=(lt == 10))
                        for ti in range(ntr):
                            for kc in kcs:
                                k0, ksz = KC[kc]
                                for pl in range(2):
                                    nc.scalar.copy(
                                        w_c[:ksz, ti, pl, kc], pss[(ti, kc, pl)][:ksz]
                                    )
                # ---- Ainv + pass7 on this column chunk ----
                with (
                    tc.tile_pool(name="atp", bufs=2) as atp,
                    tc.tile_pool(name="p7", bufs=1) as p7p,
                    tc.tile_pool(name="p7s", bufs=1) as p7s,
                    tc.tile_pool(name="psv", bufs=2, space="PSUM") as pvp,
                ):
                    for nt, (r0, p) in enumerate(NT):
                        atb = atp.tile([122, 2, 4, 128], MDT, name="atb", tag="atb")
                        for mi in range(2):
                            nc.sync.dma_start(
                                out=atb[:, mi, :, :p],
                                in_=atm[mi, :, r0 : r0 + p].rearrange(
                                    "(kc q) n -> q kc n", q=122
                                ),
                            )
                        # co planes: 0=a1u, 1=a3u, 2=skp, 3=vkp_next
                        co = p7s.tile([128, 4, 432], F32, name="co", tag="co")
                        skp = co[:p, 2]
                        hsn = None
                        if not last:
                            hsn = p7s.tile([128, 432], F32, name="hsn", tag="hsn")
                        for ti, dst in enumerate([skp] + ([hsn[:p]] if not last else [])):
                            ps = pvp.tile([128, 432], F32, name="psv", tag="psv")
                            first = True
                            for kc in range(4):
                                k0, ksz = KC[kc]
                                for pl in range(2):
                                    mm(
                                        ps[:p],
                                        atb[:ksz, pl, kc, :p],
                                        w_c[:ksz, ti, pl, kc],
                                        start=first,
                                        stop=(kc == 3 and pl == 1),
                                    )
                                    first = False
                            nc.scalar.copy(dst, ps[:p])
                        r_sl = slice(r0, r0 + p)
                        c_sl = slice(w0, w0 + wsz)
                        if last:
                            nc.sync.dma_start(
                                out=st_d[it + 1][2, r_sl, c_sl], in_=skp
                            )
                            if dbg is not None:
                                nc.sync.dma_start(
                                    out=dbg[r_sl, c_sl], in_=skp
                                )
                            continue
                        w7 = p7p.tile([128, 4, 432], F32, name="w7", tag="w7")
                        sti = w7[:p, 0:4]  # a1, a3, sk, vk of iteration `it`
                        a1t = w7[:p, 0]
                        a3t = w7[:p, 1]
                        skot = w7[:p, 2]
                        vkt = w7[:p, 3]
                        wt2 = p7p.tile([128, 3, 432], F32, name="wt2", tag="wt2")
                        ctyt = wt2[:p, 0]
                        vmt = wt2[:p, 1]
                        u2 = wt2[:p, 2]
                        if it > 0:
                            nc.sync.dma_start(
                                out=sti,
                                in_=st_d[it][:, r_sl, c_sl].rearrange(
                                    "m p j -> p m j"
                                ),
                            )
                        else:
                            nc.sync.dma_start(out=vkt, in_=st_d[0][3, r_sl, c_sl])
                        nc.sync.dma_start(out=ctyt, in_=cty[r_sl, c_sl])
                        nc.sync.dma_start(
                            out=vmt, in_=vmult[vm_idx[it + 1], r_sl, c_sl]
                        )
                        u1 = co[:p, 0]  # a1u
                        u3 = co[:p, 1]  # a3u
                        nc.vector.tensor_sub(u1, hsn[:p], vkt)
                        if it > 0:
                            nc.vector.scalar_tensor_tensor(
                                u1, u1, m1, a1t, op0=OP.mult, op1=OP.add
                            )
                        else:
                            nc.scalar.mul(u1, u1, m1)
                        if it > 0:
                            nc.gpsimd.tensor_scalar_mul(u2, skot, m3)
                            nc.gpsimd.tensor_add(u2, u2, a3t)
                            nc.scalar.activation(u2, u2, AF.Relu)
                            nc.vector.scalar_tensor_tensor(
                                u3, skp, m3, a3t, op0=OP.mult, op1=OP.add
                            )
                            nc.vector.tensor_sub(u3, u3, u2)
                        else:
                            nc.scalar.mul(u3, skp, m3)
                        vkn = co[:p, 3]
                        nc.gpsimd.tensor_scalar_mul(vkn, hsn[:p], m1n)
                        nc.gpsimd.tensor_add(vkn, vkn, u1)
                        nc.vector.tensor_add(vkn, vkn, ctyt)
                        nc.vector.tensor_mul(vkn, vkn, vmt)  # vkp_{it+1}
                        nc.sync.dma_start(
                            out=st_d[it + 1][:, r_sl, c_sl].rearrange("m p j -> p m j"),
                            in_=co[:p],
                        )
                        nc.vector.scalar_tensor_tensor(
                            vd_next[:p, nt, w0 : w0 + wsz],
                            u1,
                            -1.0 / m1n,
                            vkn,
                            op0=OP.mult,
                            op1=OP.add,
                        )
            wcp.release()

        def pass2(it, s_grid):
            _, m2, m3, tau = scalars[it]
            p_zero = it == 1
            pb_prev = (p1_d[(it - 1) % 2], p2_d[(it - 1) % 2])
            pb_cur = (p1_d[it % 2], p2_d[it % 2])
            sk_prev = st_d[it][2]
            with (
                tc.tile_pool(name="p2a_in", bufs=1) as pin,
                tc.tile_pool(name="p2a_w2", bufs=1) as pw2,
                tc.tile_pool(name="p2a_w3", bufs=1) as pw3,
            ):
                for nt, (r0, p) in enumerate(NT):
                    pS = p if r0 + p < HG else p - 1
                    w1 = pin.tile([128, 4, WG], F32, name="w1", tag="w1")
                    skT = w1[:p, 0]
                    skS = w1[:pS, 1]
                    p1o = w1[:pS, 2]
                    p2o = w1[:p, 3, 0 : WG - 1]
                    nc.sync.dma_start(out=skT, in_=sk_prev[r0 : r0 + p])
                    nc.sync.dma_start(out=skS, in_=sk_prev[r0 + 1 : r0 + 1 + pS])
                    if not p_zero:
                        nc.sync.dma_start(out=p1o, in_=pb_prev[0][r0 + 1 : r0 + 1 + pS])
                        nc.sync.dma_start(out=p2o, in_=pb_prev[1][r0 : r0 + p, 1:WG])
                    w2 = pw2.tile([128, 4, WG], F32, name="w2", tag="w2")
                    t1 = w2[:pS, 0]
                    r1 = w2[:pS, 1]
                    t2 = w2[:p, 2, 0 : WG - 1]
                    r2 = w2[:p, 3, 0 : WG - 1]
                    w3 = pw3.tile([128, 4, WG], F32, name="w3", tag="w3")
                    L1 = w3[:pS, 0]
                    sq1 = w3[:p, 1]
                    sq2 = w3[:p, 2]
                    mg = w3[:p, 3]
                    L2 = w3[:p, 3, 0 : WG - 1]  # aliases mg; L2 dead before mg
                    nc.vector.tensor_sub(L1, skT[:pS], skS)
                    nc.gpsimd.tensor_tensor(
                        L2, skT[:, 0 : WG - 1], skT[:, 1:WG], op=OP.subtract
                    )
                    if p_zero:
                        nc.scalar.mul(t1, L1, 2.0)
                        nc.gpsimd.tensor_copy(r2, L2)
                        nc.scalar.mul(t2, L2, 2.0)
                        nc.vector.tensor_copy(r1, L1)
                    else:
                        nc.vector.scalar_tensor_tensor(
                            t1, L1, 2.0, p1o, op0=OP.mult, op1=OP.add
                        )
                        nc.vector.tensor_sub(r1, t1, L1)
                        # r2 = L2 + p2o ; t2 = r2 + L2
                        nc.gpsimd.tensor_add(r2, L2, p2o)
                        nc.gpsimd.tensor_add(t2, r2, L2)
                    if pS < p:
                        nc.vector.memset(sq1, 0.0)
                    nc.scalar.square(sq1[:pS], t1)
                    nc.vector.memset(sq2, 0.0)
                    nc.scalar.square(sq2[:, 0 : WG - 1], t2)
                    nc.vector.tensor_add(sq1, sq1, sq2)  # msq
                    nc.scalar.sqrt(mg, sq1)
                    mgt = w3[:p, 1]  # reuse sq1
                    nc.scalar.activation(mgt, mg, AF.Relu, bias=-tau)
                    den = w3[:p, 2]  # reuse sq2
                    nc.scalar.activation(den, mgt, AF.Identity, bias=tau)
                    rec = w3[:p, 3]  # reuse mg
                    nc.vector.reciprocal_approx_fast(out=rec, in_=den)
                    mmlt = w3[:p, 2]  # reuse den
                    nc.vector.tensor_mul(mmlt, mgt, rec)
                    tm = w3[:p, 3]
                    nc.vector.tensor_mul(tm[:pS], t1, mmlt[:pS])
                    nc.vector.tensor_sub(r1, r1, tm[:pS])
                    nc.sync.dma_start(out=pb_cur[0][r0 + 1 : r0 + 1 + pS], in_=r1)
                    nc.gpsimd.tensor_mul(tm[:, 0 : WG - 1], t2, mmlt[:, 0 : WG - 1])
                    nc.gpsimd.tensor_tensor(
                        r2, r2, tm[:, 0 : WG - 1], op=OP.subtract
                    )
                    nc.sync.dma_start(out=pb_cur[1][r0 : r0 + p, 1:WG], in_=r2)
            with (
                tc.tile_pool(name="p2b_in", bufs=2) as pin,
                tc.tile_pool(name="p2b_w", bufs=1) as pw,
            ):
                for nt, (r0, p) in enumerate(NT):
                    w4 = pin.tile([128, 4, WG + 1], F32, name="w4", tag="w4")
                    pa = w4[:p, 0, 0:WG]
                    pb = w4[:p, 1, 0:WG]
                    p2r = w4[:p, 2]
                    skT = w4[:p, 3, 0:WG]
                    nc.sync.dma_start(out=pa, in_=pb_cur[0][r0 : r0 + p])
                    nc.sync.dma_start(out=pb, in_=pb_cur[0][r0 + 1 : r0 + 1 + p])
                    nc.sync.dma_start(out=p2r, in_=pb_cur[1][r0 : r0 + p])
                    nc.sync.dma_start(out=skT, in_=sk_prev[r0 : r0 + p])
                    a3t = pin.tile([128, WG], F32, name="a3t", tag="a3t")
                    nc.sync.dma_start(out=a3t[:p], in_=st_d[it][1, r0 : r0 + p])
                    w5 = pw.tile([128, 2, WG], F32, name="w5", tag="w5")
                    va = w5[:p, 0]
                    q = w5[:p, 1]
                    nc.vector.tensor_sub(va, pa, pb)
                    vb = w4[:p, 0, 0:WG]  # reuse pa slot
                    nc.gpsimd.tensor_tensor(
                        vb, p2r[:, 0:WG], p2r[:, 1 : WG + 1], op=OP.subtract
                    )
                    nc.vector.tensor_add(va, va, vb)  # ltv
                    nc.vector.scalar_tensor_tensor(
                        q, skT, m3, a3t[:p], op0=OP.mult, op1=OP.add
                    )
                    nc.scalar.activation(q, q, AF.Relu)  # m3*wkp
                    nc.vector.tensor_sub(q, q, a3t[:p])
                    nc.vector.scalar_tensor_tensor(
                        s_grid[:p, nt, :], va, m2, q, op0=OP.mult, op1=OP.add
                    )

        # ================= program =================

        # --- prologue: p-buffer guard zeroing ---
        with tc.tile_pool(name="zg", bufs=1) as zp:
            zt = zp.tile([128, WG], F32, name="zt")
            nc.vector.memset(zt[:], 0.0)
            for b in range(2):
                nc.sync.dma_start(out=p1_d[b][0:1], in_=zt[0:1])
                nc.sync.dma_start(out=p1_d[b][HG : HG + 1], in_=zt[0:1])
                for nt, (r0, p) in enumerate(NT):
                    nc.sync.dma_start(out=p2_d[b][r0 : r0 + p, 0:1], in_=zt[:p, 0:1])
                    nc.sync.dma_start(
                        out=p2_d[b][r0 : r0 + p, WG : WG + 1], in_=zt[:p, 0:1]
                    )

        # --- prologue: H = fft2(hps) -> hr_d/hi_d; Smult planes ---
        hgrid = big_tile([128, 8, WG], "slotA")
        nc.sync.dma_start(
            out=hgrid[:, 0:7, :], in_=hps[:896].rearrange("(t p) w -> p t w", p=128)
        )
        nc.sync.dma_start(out=hgrid[:76, 7, :], in_=hps[896:])
        ct_h = big_tile([128, 11, K2], "slotB", dt=FDT)
        with tc.tile_pool(name="fhp0", bufs=1) as fp0:
            fh0 = load_fh(fp0)
            stageA(hgrid, ct_h, fh0)

        def h_sink(lc, lsz, pss):
            l0 = WT[lc][0]
            with tc.tile_pool(name="hcp", bufs=2) as hp:
                ht = hp.tile([128, 2, KHP], F32, name="ht", tag="ht")
                nc.scalar.copy(ht[:lsz, 0], pss[(0, 0)][:lsz])
                nc.scalar.copy(ht[:lsz, 1], pss[(0, 1)][:lsz])
                nc.sync.dma_start(
                    out=hb_d[:, l0 : l0 + lsz].rearrange("m p k -> p m k"),
                    in_=ht[:lsz],
                )

        stageB([ct_h], h_sink)

        with (
            tc.tile_pool(name="smin", bufs=2) as sip,
            tc.tile_pool(name="smw", bufs=2) as swp,
        ):
            for u, (m1, m2, m3) in enumerate(sm_keys):
                for lc in range(11):
                    l0, lsz = WT[lc]
                    hin = sip.tile([128, 3, KHP], F32, name="hin", tag="hin")
                    nc.sync.dma_start(
                        out=hin[:lsz, 0:2],
                        in_=hb_d[:, l0 : l0 + lsz].rearrange("m p k -> p m k"),
                    )
                    nc.sync.dma_start(out=hin[:lsz, 2], in_=ltl[l0 : l0 + lsz])
                    sw = swp.tile([128, 2, KHP], F32, name="sw", tag="sw")
                    aa = sw[:lsz, 0]
                    bb = sw[:lsz, 1]
                    nc.vector.tensor_mul(aa, hin[:lsz, 0], hin[:lsz, 0])
                    nc.gpsimd.tensor_mul(bb, hin[:lsz, 1], hin[:lsz, 1])
                    nc.vector.tensor_add(aa, aa, bb)  # HtH
                    nc.scalar.mul(bb, hin[:lsz, 2], m2 * SCALE)
                    nc.vector.scalar_tensor_tensor(
                        aa, aa, m1 * SCALE, bb, op0=OP.mult, op1=OP.add
                    )
                    nc.scalar.activation(aa, aa, AF.Identity, bias=m3 * SCALE)
                    nc.vector.reciprocal_approx_fast(out=bb, in_=aa)
                    nc.sync.dma_start(out=sm_d[u, l0 : l0 + lsz], in_=bb)

        # --- prologue: vd_0 = vkp_0 = Vmult*Cty ---
        vd = big_tile([128, 8, WG], "slotC")
        with tc.tile_pool(name="v0in", bufs=2) as vip:
            for nt, (r0, p) in enumerate(NT):
                vin = vip.tile([128, 2, WG], F32, name="vin", tag="vin")
                nc.sync.dma_start(out=vin[:p, 0], in_=cty[r0 : r0 + p])
                nc.sync.dma_start(out=vin[:p, 1], in_=vmult[vm_idx[0], r0 : r0 + p])
                nc.vector.tensor_mul(vd[:p, nt, :], vin[:p, 0], vin[:p, 1])
                nc.sync.dma_start(
                    out=st_d[0][3, r0 : r0 + p], in_=vd[:p, nt, :].bitcast(F32)
                )

        # --- iterations; big-slot rotation: vd on V, free = {F1, F2} ---
        slots = ["slotA", "slotB", "slotC"]
        V = "slotC"
        for it in range(n_iters):
            last = it == n_iters - 1
            has_s = it > 0
            F1, F2 = [sl for sl in slots if sl != V]
            # stage A of vd first (depends only on the previous iteration),
            # so its PE work overlaps pass2's DVE/ACT chain
            with tc.tile_pool(name="fhp", bufs=1) as fp:
                fh = load_fh(fp)
                ct_v = big_tile([128, 11, K2], F1, dt=FDT)
                stageA(vd, ct_v, fh)  # vd dies here
            if has_s:
                # V-transform stage B immediately (PE work that overlaps the
                # DVE/ACT-bound pass2 below); pointwise deferred to S's stage B
                yt_v = big_tile([128, 11, K2], V, dt=FDT)
                stageB([ct_v], y_sink(yt_v))  # ct_v dies here
                s_grid = big_tile([128, 8, WG], F2)
                pass2(it, s_grid)
                with tc.tile_pool(name="fhp2", bufs=1) as fp2:
                    fh2 = load_fh(fp2)
                    ct_s = big_tile([128, 11, K2], F1, dt=FDT)
                    stageA(s_grid, ct_s, fh2)  # s dies here
                that = big_tile([128, 11, K2], F2, dt=FDT)
                stageB([ct_s], pointwise_T(it, that, True, ytv=yt_v))
            else:
                that = big_tile([128, 11, K2], F2, dt=FDT)
                stageB([ct_v], pointwise_T(it, that, False))
            ghat = None
            if not last:
                ghat = big_tile([128, 11, K2], F1, dt=FDT)
                emit_G(that, ghat)
                vd = big_tile([128, 8, WG], V)
            binv_ainv_pass7(it, that, ghat, vd if not last else None)

        # --- epilogue: crop, global max, normalize ---
        with (
            tc.tile_pool(name="ep", bufs=1) as ep,
            tc.tile_pool(name="eps", bufs=1) as eps,
        ):
            mxs = eps.tile([128, 4], F32, name="mxs")
            nc.vector.memset(mxs[:], -1e30)
            ctiles = []
            for t, (r0, p) in enumerate(CROP):
                ctile = ep.tile([128, D1], F32, name=f"ctile{t}", bufs=1)
                nc.sync.dma_start(
                    out=ctile[:p], in_=st_d[n_iters][2, r0 : r0 + p, P1 : P1 + D1]
                )
                ctiles.append(ctile)
                nc.vector.tensor_reduce(
                    out=mxs[:p, t : t + 1],
                    in_=ctile[:p],
                    axis=mybir.AxisListType.X,
                    op=mybir.AluOpType.max,
                )
            mx1 = eps.tile([128, 1], F32, name="mx1")
            nc.vector.tensor_reduce(
                out=mx1[:], in_=mxs[:], axis=mybir.AxisListType.X,
                op=mybir.AluOpType.max,
            )
            mxr = eps.tile([1, 1], F32, name="mxr")
            nc.gpsimd.tensor_reduce(
                out=mxr[:], in_=mx1[:], axis=mybir.AxisListType.C,
                op=mybir.AluOpType.max,
            )
            nc.sync.dma_start(out=ccin[:], in_=mxr[:])
            nc.gpsimd.collective_compute(
                "AllReduce",
                mybir.AluOpType.max,
                replica_groups=[[0, 1, 2, 3, 4, 5, 6, 7]],
                ins=[ccin[:]],
                outs=[ccout[:]],
            )
            gmx = eps.tile([128, 1], F32, name="gmx")
            nc.sync.dma_start(out=gmx[0:1], in_=ccout[:])
            gmxb = eps.tile([128, 1], F32, name="gmxb")
            nc.gpsimd.partition_broadcast(gmxb[:], gmx[0:1], channels=128)
            rcp = eps.tile([128, 1], F32, name="rcp")
            nc.vector.reciprocal(out=rcp[:], in_=gmxb[:])
            for t, (r0, p) in enumerate(CROP):
                o = ep.tile([128, D1], F32, name=f"o{t}", bufs=1)
                nc.scalar.activation(
                    o[:p], ctiles[t][:p], AF.Copy, bias=0.0, scale=rcp[:p]
                )
                nc.sync.dma_start(out=out[r0 - P0 : r0 - P0 + p], in_=o[:p])

        big.release()

    nc.finalize()
    return nc


def _mats_np(cw, sp, sn):
    m = np.ascontiguousarray(np.stack([cw, sp, sn]))
    if os.environ.get("K_FREQ_BF16", "0") == "1":
        import ml_dtypes

        m = m.astype(ml_dtypes.bfloat16)
    return m


_BUILD_CACHE = {}
_CONSTS = None


def _cached_consts():
    global _CONSTS
    if _CONSTS is None:
        _CONSTS = _make_consts()
    return _CONSTS


def _prepare(y, h, mu1, mu2, mu3, tau):
    y = np.asarray(y, dtype=np.float32)
    h = np.asarray(h, dtype=np.float32)
    scalars = tuple(
        (float(mu1[i]), float(mu2[i]), float(mu3[i]), float(tau[i]))
        for i in range(ITERS)
    )
    if scalars not in _BUILD_CACHE:
        _BUILD_CACHE[scalars] = _build(list(scalars))
    nc = _BUILD_CACHE[scalars]

    fhr, fhi, cw, sp, sn, atr, ati, ltl_t = _cached_consts()

    vm_keys = []
    for s in scalars:
        if s[0] not in vm_keys:
            vm_keys.append(s[0])
    ctc = np.zeros((HG, WG), np.float32)
    ctc[P0 : P0 + D0, P1 : P1 + D1] = 1.0
    vmult = np.stack([1.0 / (ctc + m1) for m1 in vm_keys]).astype(np.float32)

    hpad = np.zeros((HG, WG), np.float32)
    hpad[P0 : P0 + D0, P1 : P1 + D1] = h
    hps = np.fft.ifftshift(hpad).astype(np.float32)

    common = {
        "hps": hps,
        "vmult": vmult,
        "fhr": fhr,
        "fhi": fhi,
        "mats": _mats_np(cw, sp, sn),
        "atm": np.ascontiguousarray(np.stack([atr, ati])),
        "ltl": ltl_t,
    }
    zero_cty = np.zeros((HG, WG), np.float32)
    in_maps = []
    for c in range(8):
        m = dict(common)
        if c < 3:
            ctyc = np.zeros((HG, WG), np.float32)
            ctyc[P0 : P0 + D0, P1 : P1 + D1] = y[0, c]
            m["cty"] = ctyc
        else:
            m["cty"] = zero_cty
        in_maps.append(m)

    trace = os.environ.get("K_TRACE", "0") == "1"
    return nc, in_maps


def kernel(y, h, mu1, mu2, mu3, tau):
    from concourse.bass_utils import run_bass_kernel_spmd

    nc, in_maps = _prepare(y, h, mu1, mu2, mu3, tau)
    trace = os.environ.get("K_TRACE", "0") == "1"
    res = run_bass_kernel_spmd(nc, in_maps, core_ids=list(range(8)), trace=trace)
    kernel._exec_ns = res.exec_time_ns
    kernel._res = res
    outp = np.stack([res.results[c]["out"] for c in range(3)])[None]
    if DEBUG_SK:
        kernel._dbg = [res.results[c].get("dbg") for c in range(3)]
    return outp.astype(np.float32)
